# revision 25
# baseline (speedup 1.0000x reference)
"""Multi-head attention (B=2, S=2048, D=1024, H=16, dk=64) on 8 Trainium2
NeuronCores via Bass/Tile.

Sharding: core c handles batch b = c//4 and head-group g = c%4 (4 heads,
256 qkv columns).  Each core computes its QKV projection slices, 4 heads of
attention, and a partial output projection against its 256-row slice of Wo.
The host sums the 4 partial outputs per batch and folds in bo and bv@Wo.

v3 design (vs v2 baseline at 382us):
- Host pre-transposes x to x^T (contiguous bf16) -> plain fast DMA instead
  of the slow 2-byte DMA-transpose that stalled the PE ~30us at start.
- QKV projections run in pure bf16 (host-converted bf16 weights); scores,
  AV and out-proj in f32r.  rel_l2 budget (numpy-simulated): ~4.6e-3.
- exp split across engines: even k-chunks on ACT (exact spline exp), odd
  chunks on DVE via a one-op Schraudolph bit-trick exp
  (i32 = round(A*s + B), bits reinterpreted as fp32), halving the 147us
  serial ACT-exp wall that dominated v2.
- Software-pipelined emission: K-proj + Q(block0) first so the exp stream
  starts ~10us in; V-proj, remaining Q-proj and the out-projection are
  interleaved into the attention units as PE filler, keeping the PE dense
  so HAM stays at 2.4GHz (v2 spent 229us throttled at 1.2GHz).
- Normalization reads AV PSUM directly (no staging copy); out-proj consumes
  per-unit normalized tiles via row-tiled K=64 matmul pairs (no big oT).
"""

import numpy as np

P = 128
B, S, D = 2, 2048, 1024
H, DK = 16, 64
COLS = 256          # qkv columns per core (4 heads)
KC = D // P         # 8 contraction chunks for the projections
TT = 512            # token block (matmul free dim)
NJ = S // TT        # 4 token blocks
NKT = S // P        # 16 key-token chunks
VW = 65             # per-head AV lhsT width: 64 v-dims + ones column

# Schraudolph exp(s/8) constants: i32 = A*s + B, bitcast to fp32.
# c = 0.0547 minimizes RMS rel err (~1.8%); rel_l2 impact simulated 4.6e-3.
EXP_A = float(0.125 * 1.4426950408889634 * (1 << 23))
EXP_B = float((127 - 0.0547) * (1 << 23))
DVE_EXP = True      # some k-chunks use the DVE bit-trick exp
DVE_KCS = (1, 3, 5, 7, 9, 11, 13)   # k-chunks computed on DVE (rest on ACT)
ACT_BIAS = True     # q/k bias-add on ACT (else DVE)
ACT_VCOPY = True    # v PSUM->SBUF copy on ACT (else DVE)
ACT_OUTST = True    # even-oc out staging on ACT (else DVE)
PIPELINED = True    # interleave proj/outproj work into attention units
STAGE = 3           # 1: projections only, 2: +attention/normalize, 3: full

_CACHE = {}


def _build():
    import concourse.bass as bass
    import concourse.tile as tile
    from concourse import bacc, mybir

    f32 = mybir.dt.float32
    f32r = mybir.dt.float32r
    bf16 = mybir.dt.bfloat16
    i32 = mybir.dt.int32
    Exp = mybir.ActivationFunctionType.Exp
    Ident = mybir.ActivationFunctionType.Identity
    Mult = mybir.AluOpType.mult
    Add = mybir.AluOpType.add
    ts, ds = bass.ts, bass.ds

    nc = bacc.Bacc(
        "TRN2", target_bir_lowering=False, debug=False,
        enable_asserts=False, num_devices=8,
    )
    xt_d = nc.dram_tensor("xt", [D, S], bf16, kind="ExternalInput").ap()
    wq_d = nc.dram_tensor("wq", [D, COLS], bf16, kind="ExternalInput").ap()
    wk_d = nc.dram_tensor("wk", [D, COLS], bf16, kind="ExternalInput").ap()
    wv_d = nc.dram_tensor("wv", [D, COLS], bf16, kind="ExternalInput").ap()
    wo_d = nc.dram_tensor("wo", [COLS, D], f32, kind="ExternalInput").ap()
    bq_d = nc.dram_tensor("bq", [COLS], f32, kind="ExternalInput").ap()
    bk_d = nc.dram_tensor("bk", [COLS], f32, kind="ExternalInput").ap()
    out_d = nc.dram_tensor("out_t", [D, S], f32, kind="ExternalOutput").ap()

    with tile.TileContext(nc) as tc:
        with (
            tc.tile_pool(name="const", bufs=1) as const,
            tc.tile_pool(name="wst", bufs=1) as wst,
            tc.tile_pool(name="wpool", bufs=1) as wpool,
            tc.tile_pool(name="xpool", bufs=1) as xpool,
            tc.tile_pool(name="persist", bufs=1) as persist,
            tc.tile_pool(name="exps", bufs=3) as exps,
            tc.tile_pool(name="nstage", bufs=2) as nstage,
            tc.tile_pool(name="rstage", bufs=4) as rstage,
            tc.tile_pool(name="opool", bufs=3) as opool,
            tc.tile_pool(name="outst", bufs=4) as outst,
            tc.tile_pool(name="ps_sc", bufs=2, space="PSUM") as ps_sc,
            tc.tile_pool(name="ps_o", bufs=1, space="PSUM") as ps_o,
            tc.tile_pool(name="ps_u", bufs=2, space="PSUM") as ps_u,
        ):
            ones32 = const.tile([P, VW], f32, tag="ones32")
            nc.vector.memset(ones32[:], 1.0)
            ones_r = const.tile([P, VW], f32r, tag="ones_r")
            nc.vector.tensor_copy(ones_r[:], ones32[:])

            # DMA order favors the pre-phase critical path: first K-proj of
            # block 0 needs xth block 0 + wk + bk before anything else.
            xth = xpool.tile([P, KC, S], bf16, tag="xth", name="xth")
            xt_r = xt_d.rearrange("(o p) s -> p o s", p=P)
            nc.sync.dma_start(xth[:, :, ts(0, TT)], xt_r[:, :, ts(0, TT)])
            wk_sb = wpool.tile([P, KC, COLS], bf16, tag="wk", name="wk")
            nc.sync.dma_start(wk_sb[:], wk_d.rearrange("(o p) f -> p o f", p=P))
            bk_sb = const.tile([P, 2], f32, tag="bk")
            nc.sync.dma_start(bk_sb[:], bk_d.rearrange("(o p) -> p o", p=P))
            for j in range(1, NJ):
                nc.sync.dma_start(xth[:, :, ts(j, TT)], xt_r[:, :, ts(j, TT)])
            wq_sb = wpool.tile([P, KC, COLS], bf16, tag="wq", name="wq")
            nc.sync.dma_start(wq_sb[:], wq_d.rearrange("(o p) f -> p o f", p=P))
            bq_sb = const.tile([P, 2], f32, tag="bq")
            nc.sync.dma_start(bq_sb[:], bq_d.rearrange("(o p) -> p o", p=P))
            wv_sb = wpool.tile([P, KC, COLS], bf16, tag="wv", name="wv")
            nc.sync.dma_start(wv_sb[:], wv_d.rearrange("(o p) f -> p o f", p=P))
            wo_st = wst.tile([P, 2, D], f32, tag="wost", name="wost")
            nc.sync.dma_start(wo_st[:], wo_d.rearrange("(o p) f -> p o f", p=P))
            wo_r = wpool.tile([P, 2, D], f32r, tag="wo")
            nc.vector.tensor_copy(wo_r[:], wo_st[:])

            # persistent activations (attention pipeline in bf16)
            qT = persist.tile([P, 2, S], bf16, tag="qT")    # [qcol, tok]
            kT = persist.tile([P, 2, S], bf16, tag="kT")    # [kcol, tok]
            vt = persist.tile([P, NKT, 4 * VW], bf16, tag="vt")  # [tok, h*(V|1)]

            # ones column (index 64 of each head's VW slice)
            vt_heads = vt[:].rearrange("p t (h c) -> p t h c", c=VW)
            nc.vector.tensor_copy(
                vt_heads[:, :, :, 64],
                ones32[:, : NKT * 4].rearrange("p (t h) -> p t h", h=4),
            )

            # ---------------- projection emitters ----------------
            def qk_group(j, ct, wmat, bsb, dstT, pool, tag):
                acc = pool.tile([P, TT], f32, tag=tag, name="qk_acc")
                for kc in range(KC):
                    nc.tensor.matmul(
                        acc[:], wmat[:, kc, ts(ct, P)], xth[:, kc, ts(j, TT)],
                        start=(kc == 0), stop=(kc == KC - 1),
                    )
                if ACT_BIAS:
                    nc.scalar.activation(
                        dstT[:, ct, ts(j, TT)], acc[:], Ident,
                        bias=bsb[:, ct : ct + 1],
                    )
                else:
                    nc.vector.tensor_scalar_add(
                        dstT[:, ct, ts(j, TT)], acc[:], bsb[:, ct : ct + 1],
                    )

            def v_group(g):
                vacc = ps_u.tile([P, 2, COLS], f32, tag="u", name="v_acc")
                for t in range(2):
                    tt = 2 * g + t
                    for kc in range(KC):
                        nc.tensor.matmul(
                            vacc[:, t, :], xth[:, kc, ds(tt * P, P)],
                            wv_sb[:, kc, :],
                            start=(kc == 0), stop=(kc == KC - 1),
                        )
                eng = nc.scalar.copy if ACT_VCOPY else nc.vector.tensor_copy
                eng(
                    vt_heads[:, 2 * g : 2 * g + 2, :, 0:64],
                    vacc[:].rearrange("p t (h c) -> p t h c", c=64),
                )

            # ---------------- attention unit ----------------
            onrm2_of = {}

            def attention_unit(j, p, fillers):
                o2 = ps_o.tile([P, 2, TT], f32, tag="o2", name="o2")
                onrm2 = opool.tile([P, TT], f32r, tag="onrm2", name="onrm2")
                onrm2_of[(j, p)] = onrm2
                sc_tiles = {}

                def sc_pair(kc):
                    t = ps_sc.tile([P, 2, TT], f32, tag="sc", name="sc")
                    sc_tiles[kc] = t
                    for i in range(2):
                        lo = 64 * i
                        nc.tensor.matmul(
                            t[:, i, :],
                            kT[lo : lo + 64, p, ts(kc, P)],
                            qT[lo : lo + 64, p, ts(j, TT)],
                            start=True, stop=True,
                        )

                def exp_emit(kc):
                    src = sc_tiles.pop(kc)
                    ex = exps.tile([P, 2, TT], bf16, tag="ex", name="ex")
                    if (kc not in DVE_KCS) or not DVE_EXP:
                        nc.scalar.activation(ex[:], src[:], Exp, scale=0.125)
                    else:
                        # Schraudolph bit-trick on DVE: i32 = A*s + B; the
                        # fp32-bit view is converted to bf16 on gpsimd,
                        # which is otherwise idle.
                        exi = exps.tile([P, 2, TT], i32, tag="exi", name="exi")
                        nc.vector.tensor_scalar(
                            exi[:], src[:], EXP_A, EXP_B, Mult, Add,
                        )
                        nc.gpsimd.tensor_copy(ex[:], exi[:].bitcast(f32))
                    return ex

                def av_emit(kc, ex):
                    for i in range(2):
                        h = 2 * p + i
                        nc.tensor.matmul(
                            o2[0:VW, i, :],
                            vt[:, kc, ds(VW * h, VW)],
                            ex[:, i, :],
                            start=(kc == 0), stop=(kc == NKT - 1),
                        )

                sc_pair(0)
                sc_pair(1)
                prev = None
                for kc in range(NKT):
                    for f in fillers.get(kc, ()):
                        f()
                    ex = exp_emit(kc)
                    if prev is not None:
                        av_emit(kc - 1, prev)
                    # emitted after exp(kc): the sc pool buffer this pair
                    # reuses is the one exp(kc) just read (bufs=2 rotation)
                    if kc + 2 < NKT:
                        sc_pair(kc + 2)
                    prev = ex
                av_emit(NKT - 1, prev)

                # normalize: sums row -> PE broadcast -> recip -> scale
                nsums = nstage.tile([P, 2, TT], f32r, tag="nsums", name="ns")
                nc.vector.tensor_copy(nsums[64:65, :, :], o2[64:65, :, :])
                for i in range(2):
                    rbc = ps_u.tile([64, TT], f32, tag="u", name="rbc")
                    nc.tensor.matmul(
                        rbc[:], ones_r[64:65, 0:64], nsums[64:65, i, :],
                        start=True, stop=True,
                    )
                    rbs = rstage.tile([64, TT], f32, tag="rbs", name="rbs")
                    nc.vector.reciprocal_approx_fast(rbs[:], rbc[:])
                    if i == 0:
                        nc.vector.tensor_tensor(
                            onrm2[0:64, :], o2[0:64, 0, :], rbs[:], Mult,
                        )
                    else:
                        ntmp = rstage.tile([64, TT], f32r, tag="ntmp",
                                           name="ntmp")
                        nc.vector.tensor_tensor(
                            ntmp[:], o2[0:64, 1, :], rbs[:], Mult,
                        )
                        nc.sync.dma_start(onrm2[64:128, :], ntmp[:])

            # ---------------- output projection ----------------
            def wo_pair(j, oc0):
                for oc in (oc0, oc0 + 1):
                    wacc = ps_u.tile([P, TT], f32, tag="u", name="wo_acc")
                    for vc in range(2):
                        nc.tensor.matmul(
                            wacc[:], wo_r[:, vc, ds(P * oc, P)],
                            onrm2_of[(j, vc)][:],
                            start=(vc == 0), stop=(vc == 1),
                        )
                    st = outst.tile([P, TT], f32, tag="outst", name="outst")
                    if ACT_OUTST and oc % 2 == 0:
                        nc.scalar.copy(st[:], wacc[:])
                    else:
                        nc.vector.tensor_copy(st[:], wacc[:])
                    nc.sync.dma_start(out_d[ts(oc, P), ts(j, TT)], st[:])

            # ---------------- emission schedule ----------------
            # pre-phase: K projection (all blocks) + Q(block 0, ct 0)
            for j in range(NJ):
                for ct in range(2):
                    qk_group(j, ct, wk_sb, bk_sb, kT, ps_u, "u")
            qk_group(0, 0, wq_sb, bq_sb, qT, ps_u, "u")

            Fq = lambda j, ct: (
                lambda: qk_group(j, ct, wq_sb, bq_sb, qT, ps_u, "u")
            )
            Fv = lambda g: (lambda: v_group(g))
            Fo = lambda j, oc0: (lambda: wo_pair(j, oc0))

            if PIPELINED:
                fillers = {
                    0: {0: [Fq(0, 1), Fv(0)], 1: [Fv(1)], 3: [Fv(2)],
                        5: [Fv(3)], 7: [Fv(4)], 9: [Fv(5)], 11: [Fv(6)],
                        13: [Fv(7)]},
                    1: {0: [Fq(1, 0)], 2: [Fq(1, 1)]},
                    2: {0: [Fo(0, 0)], 1: [Fo(0, 2)], 2: [Fo(0, 4)],
                        3: [Fo(0, 6)], 4: [Fq(2, 0)]},
                    3: {0: [Fq(2, 1)]},
                    4: {0: [Fo(1, 0)], 1: [Fo(1, 2)], 2: [Fo(1, 4)],
                        3: [Fo(1, 6)], 4: [Fq(3, 0)]},
                    5: {0: [Fq(3, 1)]},
                    6: {0: [Fo(2, 0)], 2: [Fo(2, 2)]},
                    7: {0: [Fo(2, 4)], 2: [Fo(2, 6)]},
                }
                for u in range(8):
                    attention_unit(u // 2, u % 2, fillers[u])
                for oc0 in (0, 2, 4, 6):
                    wo_pair(3, oc0)
            else:
                qk_group(0, 1, wq_sb, bq_sb, qT, ps_u, "u")
                for j in range(1, NJ):
                    for ct in range(2):
                        qk_group(j, ct, wq_sb, bq_sb, qT, ps_u, "u")
                for g in range(8):
                    v_group(g)
                if STAGE == 1:
                    # dump a kT/qT/vt slice through outst so out_t is written
                    for j in range(NJ):
                        st = outst.tile([P, TT], f32, tag="outst", name="o1")
                        nc.vector.tensor_copy(st[:], kT[:, 0, ts(j, TT)])
                        nc.sync.dma_start(out_d[0:P, ts(j, TT)], st[:])
                        st2 = outst.tile([P, TT], f32, tag="outst", name="o2s")
                        nc.vector.tensor_copy(st2[:], qT[:, 1, ts(j, TT)])
                        nc.sync.dma_start(out_d[P : 2 * P, ts(j, TT)], st2[:])
                else:
                    for u in range(8):
                        attention_unit(u // 2, u % 2, {})
                        if STAGE >= 3 and u % 2 == 1:
                            for oc0 in (0, 2, 4, 6):
                                wo_pair(u // 2, oc0)
                    if STAGE == 2:
                        for jp, t in onrm2_of.items():
                            st = outst.tile([P, TT], f32, tag="outst",
                                            name="o2d")
                            nc.vector.tensor_copy(st[:], t[:])
                            nc.sync.dma_start(
                                out_d[ts(jp[1], P), ts(jp[0], TT)], st[:]
                            )

    nc.compile()
    return nc


def make_in_maps(x, Wq, bq, Wk, bk, Wv, Wo):
    import ml_dtypes

    bf = ml_dtypes.bfloat16
    xt = [np.ascontiguousarray(x[b].T.astype(bf)) for b in range(B)]

    in_maps = []
    for c in range(8):
        b, g = divmod(c, 4)
        cs = slice(COLS * g, COLS * (g + 1))
        in_maps.append({
            "xt": xt[b],
            "wq": np.ascontiguousarray(Wq[:, cs].astype(bf)),
            "wk": np.ascontiguousarray(Wk[:, cs].astype(bf)),
            "wv": np.ascontiguousarray(Wv[:, cs].astype(bf)),
            "wo": np.ascontiguousarray(Wo[cs, :]),
            "bq": np.ascontiguousarray(bq[cs]),
            "bk": np.ascontiguousarray(bk[cs]),
        })
    return in_maps


def kernel(x, Wq, bq, Wk, bk, Wv, bv, Wo, bo):
    from concourse import bass_utils

    x = np.asarray(x, dtype=np.float32)
    Wq = np.asarray(Wq, dtype=np.float32)
    Wk = np.asarray(Wk, dtype=np.float32)
    Wv = np.asarray(Wv, dtype=np.float32)
    Wo = np.asarray(Wo, dtype=np.float32)
    bq = np.asarray(bq, dtype=np.float32)
    bk = np.asarray(bk, dtype=np.float32)
    bv = np.asarray(bv, dtype=np.float32)
    bo = np.asarray(bo, dtype=np.float32)

    if "nc" not in _CACHE:
        _CACHE["nc"] = _build()
    nc = _CACHE["nc"]

    in_maps = make_in_maps(x, Wq, bq, Wk, bk, Wv, Wo)
    res = bass_utils.run_bass_kernel_spmd(nc, in_maps, core_ids=list(range(8)))

    out = np.zeros((B, S, D), dtype=np.float32)
    for c in range(8):
        out[c // 4] += res.results[c]["out_t"].T
    out += bo + bv @ Wo
    return out


# revision 31
# speedup vs baseline: 1.3634x; 1.3634x over previous
"""Multi-head attention (B=2, S=2048, D=1024, H=16, dk=64) on 8 Trainium2
NeuronCores via Bass/Tile.

Sharding: core c handles batch b = c//4 and head-group g = c%4 (4 heads,
256 qkv columns).  Each core computes its QKV projection slices, 4 heads of
attention, and a partial output projection against its 256-row slice of Wo.
The host sums the 4 partial outputs per batch and folds in bo and bv@Wo.

v3 design (vs v2 baseline at 382us):
- Host pre-transposes x to x^T (contiguous bf16) -> plain fast DMA instead
  of the slow 2-byte DMA-transpose that stalled the PE ~30us at start.
- QKV projections run in pure bf16 (host-converted bf16 weights); scores,
  AV and out-proj in f32r.  rel_l2 budget (numpy-simulated): ~4.6e-3.
- exp split across engines: even k-chunks on ACT (exact spline exp), odd
  chunks on DVE via a one-op Schraudolph bit-trick exp
  (i32 = round(A*s + B), bits reinterpreted as fp32), halving the 147us
  serial ACT-exp wall that dominated v2.
- Software-pipelined emission: K-proj + Q(block0) first so the exp stream
  starts ~10us in; V-proj, remaining Q-proj and the out-projection are
  interleaved into the attention units as PE filler, keeping the PE dense
  so HAM stays at 2.4GHz (v2 spent 229us throttled at 1.2GHz).
- Normalization reads AV PSUM directly (no staging copy); out-proj consumes
  per-unit normalized tiles via row-tiled K=64 matmul pairs (no big oT).
"""

import numpy as np

P = 128
B, S, D = 2, 2048, 1024
H, DK = 16, 64
COLS = 256          # qkv columns per core (4 heads)
KC = D // P         # 8 contraction chunks for the projections
TT = 512            # token block (matmul free dim)
NJ = S // TT        # 4 token blocks
NKT = S // P        # 16 key-token chunks
VW = 65             # per-head AV lhsT width: 64 v-dims + ones column

# Schraudolph exp(s/8) constants: i32 = A*s + B, bitcast to fp32.
# c = 0.0547 minimizes RMS rel err (~1.8%); rel_l2 impact simulated 4.6e-3.
EXP_A = float(0.125 * 1.4426950408889634 * (1 << 23))
EXP_B = float((127 - 0.0547) * (1 << 23))
DVE_EXP = True      # some k-chunks use the DVE bit-trick exp
DVE_KCS = (1, 4, 6, 9, 11, 14)      # k-chunks computed on DVE (rest on ACT)
ACT_BIAS = True     # q/k bias-add on ACT (else DVE)
ACT_VCOPY = True    # v PSUM->SBUF copy on ACT (else DVE)
ACT_OUTST = True    # even-oc out staging on ACT (else DVE)
PIPELINED = True    # interleave proj/outproj work into attention units
STAGE = 3           # 1: projections only, 2: +attention/normalize, 3: full

_CACHE = {}


def _build():
    import concourse.bass as bass
    import concourse.tile as tile
    from concourse import bacc, mybir

    f32 = mybir.dt.float32
    f32r = mybir.dt.float32r
    bf16 = mybir.dt.bfloat16
    i32 = mybir.dt.int32
    Exp = mybir.ActivationFunctionType.Exp
    Ident = mybir.ActivationFunctionType.Identity
    Mult = mybir.AluOpType.mult
    Add = mybir.AluOpType.add
    ts, ds = bass.ts, bass.ds

    nc = bacc.Bacc(
        "TRN2", target_bir_lowering=False, debug=False,
        enable_asserts=False, num_devices=8,
    )
    xt_d = nc.dram_tensor("xt", [D, S], bf16, kind="ExternalInput").ap()
    wq_d = nc.dram_tensor("wq", [D, COLS], bf16, kind="ExternalInput").ap()
    wk_d = nc.dram_tensor("wk", [D, COLS], bf16, kind="ExternalInput").ap()
    wv_d = nc.dram_tensor("wv", [D, COLS], bf16, kind="ExternalInput").ap()
    wo_d = nc.dram_tensor("wo", [COLS, D], f32, kind="ExternalInput").ap()
    bq_d = nc.dram_tensor("bq", [COLS], f32, kind="ExternalInput").ap()
    bk_d = nc.dram_tensor("bk", [COLS], f32, kind="ExternalInput").ap()
    out_d = nc.dram_tensor("out_t", [D, S], f32, kind="ExternalOutput").ap()

    with tile.TileContext(nc) as tc:
        with (
            tc.tile_pool(name="const", bufs=1) as const,
            tc.tile_pool(name="wst", bufs=1) as wst,
            tc.tile_pool(name="wpool", bufs=1) as wpool,
            tc.tile_pool(name="xpool", bufs=1) as xpool,
            tc.tile_pool(name="persist", bufs=1) as persist,
            tc.tile_pool(name="exps", bufs=3) as exps,
            tc.tile_pool(name="nstage", bufs=2) as nstage,
            tc.tile_pool(name="rstage", bufs=4) as rstage,
            tc.tile_pool(name="opool", bufs=3) as opool,
            tc.tile_pool(name="outst", bufs=4) as outst,
            tc.tile_pool(name="ps_sc", bufs=2, space="PSUM") as ps_sc,
            tc.tile_pool(name="ps_o", bufs=1, space="PSUM") as ps_o,
            tc.tile_pool(name="ps_u", bufs=2, space="PSUM") as ps_u,
        ):
            ones32 = const.tile([P, VW], f32, tag="ones32")
            nc.vector.memset(ones32[:], 1.0)
            ones_r = const.tile([P, VW], f32r, tag="ones_r")
            nc.vector.tensor_copy(ones_r[:], ones32[:])

            # DMA order favors the pre-phase critical path: first K-proj of
            # block 0 needs xth block 0 + wk + bk before anything else.
            xth = xpool.tile([P, KC, S], bf16, tag="xth", name="xth")
            xt_r = xt_d.rearrange("(o p) s -> p o s", p=P)
            nc.sync.dma_start(xth[:, :, ts(0, TT)], xt_r[:, :, ts(0, TT)])
            wk_sb = wpool.tile([P, KC, COLS], bf16, tag="wk", name="wk")
            nc.sync.dma_start(wk_sb[:], wk_d.rearrange("(o p) f -> p o f", p=P))
            bk_sb = const.tile([P, 2], f32, tag="bk")
            nc.sync.dma_start(bk_sb[:], bk_d.rearrange("(o p) -> p o", p=P))
            for j in range(1, NJ):
                nc.sync.dma_start(xth[:, :, ts(j, TT)], xt_r[:, :, ts(j, TT)])
            wq_sb = wpool.tile([P, KC, COLS], bf16, tag="wq", name="wq")
            nc.sync.dma_start(wq_sb[:], wq_d.rearrange("(o p) f -> p o f", p=P))
            bq_sb = const.tile([P, 2], f32, tag="bq")
            nc.sync.dma_start(bq_sb[:], bq_d.rearrange("(o p) -> p o", p=P))
            wv_sb = wpool.tile([P, KC, COLS], bf16, tag="wv", name="wv")
            nc.sync.dma_start(wv_sb[:], wv_d.rearrange("(o p) f -> p o f", p=P))
            wo_st = wst.tile([P, 2, D], f32, tag="wost", name="wost")
            nc.sync.dma_start(wo_st[:], wo_d.rearrange("(o p) f -> p o f", p=P))
            wo_r = wpool.tile([P, 2, D], f32r, tag="wo")
            nc.vector.tensor_copy(wo_r[:], wo_st[:])

            # persistent activations (attention pipeline in bf16)
            qT = persist.tile([P, 2, S], bf16, tag="qT")    # [qcol, tok]
            kT = persist.tile([P, 2, S], bf16, tag="kT")    # [kcol, tok]
            vt = persist.tile([P, NKT, 4 * VW], bf16, tag="vt")  # [tok, h*(V|1)]

            # ones column (index 64 of each head's VW slice)
            vt_heads = vt[:].rearrange("p t (h c) -> p t h c", c=VW)
            nc.vector.tensor_copy(
                vt_heads[:, :, :, 64],
                ones32[:, : NKT * 4].rearrange("p (t h) -> p t h", h=4),
            )

            # ---------------- projection emitters ----------------
            def qk_group(j, ct, wmat, bsb, dstT, pool, tag):
                acc = pool.tile([P, TT], f32, tag=tag, name="qk_acc")
                for kc in range(KC):
                    nc.tensor.matmul(
                        acc[:], wmat[:, kc, ts(ct, P)], xth[:, kc, ts(j, TT)],
                        start=(kc == 0), stop=(kc == KC - 1),
                    )
                if ACT_BIAS:
                    nc.scalar.activation(
                        dstT[:, ct, ts(j, TT)], acc[:], Ident,
                        bias=bsb[:, ct : ct + 1],
                    )
                else:
                    nc.vector.tensor_scalar_add(
                        dstT[:, ct, ts(j, TT)], acc[:], bsb[:, ct : ct + 1],
                    )

            def v_group(g):
                vacc = ps_u.tile([P, 2, COLS], f32, tag="u", name="v_acc")
                for t in range(2):
                    tt = 2 * g + t
                    for kc in range(KC):
                        nc.tensor.matmul(
                            vacc[:, t, :], xth[:, kc, ds(tt * P, P)],
                            wv_sb[:, kc, :],
                            start=(kc == 0), stop=(kc == KC - 1),
                        )
                eng = nc.scalar.copy if ACT_VCOPY else nc.vector.tensor_copy
                eng(
                    vt_heads[:, 2 * g : 2 * g + 2, :, 0:64],
                    vacc[:].rearrange("p t (h c) -> p t h c", c=64),
                )

            # ---------------- attention unit ----------------
            onrm2_of = {}

            def attention_unit(j, p, fillers, pending=()):
                sc_tiles = {}
                o2 = None
                onrm2 = opool.tile([P, TT], f32r, tag="onrm2", name="onrm2")
                onrm2_of[(j, p)] = onrm2

                def sc_pair(kc):
                    t = ps_sc.tile([P, 2, TT], f32, tag="sc", name="sc")
                    sc_tiles[kc] = t
                    for i in range(2):
                        lo = 64 * i
                        nc.tensor.matmul(
                            t[:, i, :],
                            kT[lo : lo + 64, p, ts(kc, P)],
                            qT[lo : lo + 64, p, ts(j, TT)],
                            start=True, stop=True,
                        )

                def exp_emit(kc):
                    src = sc_tiles.pop(kc)
                    ex = exps.tile([P, 2, TT], bf16, tag="ex", name="ex")
                    if (kc not in DVE_KCS) or not DVE_EXP:
                        nc.scalar.activation(ex[:], src[:], Exp, scale=0.125)
                    else:
                        # Schraudolph bit-trick on DVE: i32 = A*s + B; the
                        # fp32-bit view is converted to bf16 on gpsimd,
                        # which is otherwise idle.
                        exi = exps.tile([P, 2, TT], i32, tag="exi", name="exi")
                        nc.vector.tensor_scalar(
                            exi[:], src[:], EXP_A, EXP_B, Mult, Add,
                        )
                        nc.vector.tensor_copy(ex[:], exi[:].bitcast(f32))
                    return ex

                def av_emit(kc, ex):
                    for i in range(2):
                        h = 2 * p + i
                        nc.tensor.matmul(
                            o2[0:VW, i, :],
                            vt[:, kc, ds(VW * h, VW)],
                            ex[:, i, :],
                            start=(kc == 0), stop=(kc == NKT - 1),
                        )

                sc_pair(0)
                sc_pair(1)
                # previous unit's normalize lands here: its DVE/ACT chain
                # overlaps this unit's first scores/exp instead of stalling
                # the PE at the unit boundary
                for f in pending:
                    f()
                o2 = ps_o.tile([P, 2, TT], f32, tag="o2", name="o2")
                prev = None
                for kc in range(NKT):
                    for f in fillers.get(kc, ()):
                        f()
                    ex = exp_emit(kc)
                    if prev is not None:
                        av_emit(kc - 1, prev)
                    # emitted after exp(kc): the sc pool buffer this pair
                    # reuses is the one exp(kc) just read (bufs=2 rotation)
                    if kc + 2 < NKT:
                        sc_pair(kc + 2)
                    prev = ex
                av_emit(NKT - 1, prev)

                def normalize():
                    # sums row -> PE broadcast -> recip -> scale
                    nsums = nstage.tile([P, 2, TT], f32r, tag="nsums",
                                        name="ns")
                    nc.scalar.copy(nsums[64:65, :, :], o2[64:65, :, :])
                    for i in range(2):
                        rbc = ps_u.tile([64, TT], f32, tag="u", name="rbc")
                        nc.tensor.matmul(
                            rbc[:], ones_r[64:65, 0:64], nsums[64:65, i, :],
                            start=True, stop=True,
                        )
                        rbs = rstage.tile([64, TT], f32, tag="rbs", name="rbs")
                        nc.vector.reciprocal_approx_fast(rbs[:], rbc[:])
                        if i == 0:
                            nc.vector.tensor_tensor(
                                onrm2[0:64, :], o2[0:64, 0, :], rbs[:], Mult,
                            )
                        else:
                            ntmp = rstage.tile([64, TT], f32r, tag="ntmp",
                                               name="ntmp")
                            nc.vector.tensor_tensor(
                                ntmp[:], o2[0:64, 1, :], rbs[:], Mult,
                            )
                            nc.sync.dma_start(onrm2[64:128, :], ntmp[:])

                return normalize

            # ---------------- output projection ----------------
            def wo_pair(j, oc0):
                for oc in (oc0, oc0 + 1):
                    wacc = ps_u.tile([P, TT], f32, tag="u", name="wo_acc")
                    for vc in range(2):
                        nc.tensor.matmul(
                            wacc[:], wo_r[:, vc, ds(P * oc, P)],
                            onrm2_of[(j, vc)][:],
                            start=(vc == 0), stop=(vc == 1),
                        )
                    st = outst.tile([P, TT], f32, tag="outst", name="outst")
                    if ACT_OUTST and oc % 2 == 0:
                        nc.scalar.copy(st[:], wacc[:])
                    else:
                        nc.vector.tensor_copy(st[:], wacc[:])
                    nc.sync.dma_start(out_d[ts(oc, P), ts(j, TT)], st[:])

            # ---------------- emission schedule ----------------
            # pre-phase: K projection (all blocks) + Q(block 0, ct 0)
            for j in range(NJ):
                for ct in range(2):
                    qk_group(j, ct, wk_sb, bk_sb, kT, ps_u, "u")
            qk_group(0, 0, wq_sb, bq_sb, qT, ps_u, "u")

            Fq = lambda j, ct: (
                lambda: qk_group(j, ct, wq_sb, bq_sb, qT, ps_u, "u")
            )
            Fv = lambda g: (lambda: v_group(g))
            Fo = lambda j, oc0: (lambda: wo_pair(j, oc0))

            if PIPELINED:
                fillers = {
                    0: {0: [Fq(0, 1), Fv(0)], 1: [Fv(1)], 3: [Fv(2)],
                        5: [Fv(3)], 7: [Fv(4)], 9: [Fv(5)], 11: [Fv(6)],
                        13: [Fv(7)]},
                    1: {0: [Fq(1, 0)], 2: [Fq(1, 1)]},
                    2: {0: [Fo(0, 0)], 1: [Fo(0, 2)], 2: [Fo(0, 4)],
                        3: [Fo(0, 6)], 4: [Fq(2, 0)]},
                    3: {0: [Fq(2, 1)]},
                    4: {0: [Fo(1, 0)], 1: [Fo(1, 2)], 2: [Fo(1, 4)],
                        3: [Fo(1, 6)], 4: [Fq(3, 0)]},
                    5: {0: [Fq(3, 1)]},
                    6: {0: [Fo(2, 0)], 2: [Fo(2, 2)]},
                    7: {0: [Fo(2, 4)], 2: [Fo(2, 6)]},
                }
                norm_fn = None
                for u in range(8):
                    attention_unit_pending = (norm_fn,) if norm_fn else ()
                    norm_fn = attention_unit(
                        u // 2, u % 2, fillers[u],
                        pending=attention_unit_pending,
                    )
                norm_fn()
                for oc0 in (0, 2, 4, 6):
                    wo_pair(3, oc0)
            else:
                qk_group(0, 1, wq_sb, bq_sb, qT, ps_u, "u")
                for j in range(1, NJ):
                    for ct in range(2):
                        qk_group(j, ct, wq_sb, bq_sb, qT, ps_u, "u")
                for g in range(8):
                    v_group(g)
                if STAGE == 1:
                    # dump a kT/qT/vt slice through outst so out_t is written
                    for j in range(NJ):
                        st = outst.tile([P, TT], f32, tag="outst", name="o1")
                        nc.vector.tensor_copy(st[:], kT[:, 0, ts(j, TT)])
                        nc.sync.dma_start(out_d[0:P, ts(j, TT)], st[:])
                        st2 = outst.tile([P, TT], f32, tag="outst", name="o2s")
                        nc.vector.tensor_copy(st2[:], qT[:, 1, ts(j, TT)])
                        nc.sync.dma_start(out_d[P : 2 * P, ts(j, TT)], st2[:])
                else:
                    for u in range(8):
                        norm_fn = attention_unit(u // 2, u % 2, {})
                        norm_fn()
                        if STAGE >= 3 and u % 2 == 1:
                            for oc0 in (0, 2, 4, 6):
                                wo_pair(u // 2, oc0)
                    if STAGE == 2:
                        for jp, t in onrm2_of.items():
                            st = outst.tile([P, TT], f32, tag="outst",
                                            name="o2d")
                            nc.vector.tensor_copy(st[:], t[:])
                            nc.sync.dma_start(
                                out_d[ts(jp[1], P), ts(jp[0], TT)], st[:]
                            )

    nc.compile()
    return nc


def make_in_maps(x, Wq, bq, Wk, bk, Wv, Wo):
    import ml_dtypes

    bf = ml_dtypes.bfloat16
    xt = [np.ascontiguousarray(x[b].T.astype(bf)) for b in range(B)]

    in_maps = []
    for c in range(8):
        b, g = divmod(c, 4)
        cs = slice(COLS * g, COLS * (g + 1))
        in_maps.append({
            "xt": xt[b],
            "wq": np.ascontiguousarray(Wq[:, cs].astype(bf)),
            "wk": np.ascontiguousarray(Wk[:, cs].astype(bf)),
            "wv": np.ascontiguousarray(Wv[:, cs].astype(bf)),
            "wo": np.ascontiguousarray(Wo[cs, :]),
            "bq": np.ascontiguousarray(bq[cs]),
            "bk": np.ascontiguousarray(bk[cs]),
        })
    return in_maps


def kernel(x, Wq, bq, Wk, bk, Wv, bv, Wo, bo):
    from concourse import bass_utils

    x = np.asarray(x, dtype=np.float32)
    Wq = np.asarray(Wq, dtype=np.float32)
    Wk = np.asarray(Wk, dtype=np.float32)
    Wv = np.asarray(Wv, dtype=np.float32)
    Wo = np.asarray(Wo, dtype=np.float32)
    bq = np.asarray(bq, dtype=np.float32)
    bk = np.asarray(bk, dtype=np.float32)
    bv = np.asarray(bv, dtype=np.float32)
    bo = np.asarray(bo, dtype=np.float32)

    if "nc" not in _CACHE:
        _CACHE["nc"] = _build()
    nc = _CACHE["nc"]

    in_maps = make_in_maps(x, Wq, bq, Wk, bk, Wv, Wo)
    res = bass_utils.run_bass_kernel_spmd(nc, in_maps, core_ids=list(range(8)))

    out = np.zeros((B, S, D), dtype=np.float32)
    for c in range(8):
        out[c // 4] += res.results[c]["out_t"].T
    out += bo + bv @ Wo
    return out
